# revision 1
# baseline (speedup 1.0000x reference)
"""GAT layer (4096 nodes, 8 heads, 64 feat/head) on 8 Trainium2 NeuronCores.

Sharding: node dim i (rows of x/adj/output) split 8 ways; W/a replicated;
each core computes attention+aggregation for its 512-row block against all
4096 j-nodes (Wh computed redundantly per core from the full x).

Math (per core, per head h):
  s_i = e_src[i,h], d_j = e_dst[j,h]   (from x @ (W @ A_{src,dst}), exact fp32)
  exp(lrelu(s+d)) = max(exp(0.2(s+d)), exp(s+d))
                  = exp(0.2 s_i) * max(q_j, w[j,i]),  q_j = exp(0.2 d_j),
                    w[j,i] = exp(0.8 s_i + d_j)
  The common factor exp(0.2 s_i) cancels in the softmax ratio, so
  g[j,i] = adj[i,j] * max(q_j, w[j,i]) is the unnormalized attention and
  out[i,d] = relu( (sum_j g[j,i] Wh[j,d]) / (sum_j g[j,i]) ),
  both sums from ONE f32r matmul per (j-chunk, head) with rhs=g and
  lhsT=[Wh_h | ones] (65 cols).
Per [128j x 512i] tile (path-split to balance engines):
  P1 tiles: ACT exp (scale/bias) + fused DVE scalar_tensor_tensor
            (max then mult by the bf16 adjacency mask)
  P3 tiles: w further factored as E8[h]*exp(d_j) with E8 = exp(0.8 s)
            precomputed once per head -> dual-op DVE tensor_scalar
            (mult scalar, max scalar; 2x mode) + mask-mult batched in
            pairs on GPSIMD (amortizes its per-op launch overhead)
then one f32r (tf32) matmul per tile accumulating [Wh_h | 1]^T @ g into
PSUM over the 32 j-chunks; finalize transposes + relu(num*recip(den)).
f32r keeps ~1.4e-3 max rel error (set GAT_EXACT=1 for full-fp32
matmuls at ~5e-6 and ~1.6x the runtime).
"""

import sys

sys.path.insert(0, "/opt/trn_rl_repo")

import ml_dtypes
import numpy as np

import concourse.bass as bass
import concourse.mybir as mybir
import concourse.tile as tile
from concourse import bacc
from concourse.bass_utils import run_bass_kernel_spmd
from concourse.masks import make_identity

N_NODES = 4096
IN_FEAT = 256
OUT_FEAT = 64
N_HEADS = 8
N_CORES = 8
ROWS = N_NODES // N_CORES          # 512 i-rows per core
F = N_HEADS * OUT_FEAT             # 512
JCH = N_NODES // 128               # 32 j-chunks
KCH = IN_FEAT // 128               # 2 k-chunks
E = OUT_FEAT + 1                   # 65: per-head lhsT cols (Wh | ones)

f32 = mybir.dt.float32
f32r = mybir.dt.float32r
bf16 = mybir.dt.bfloat16
AF = mybir.ActivationFunctionType
OP = mybir.AluOpType


def build_nc(wh_f32r=True, agg_f32r=True, repeat=1, p1_mod=9, p1_all=0,
             wh1_act_copies=False, ev_act_copies=True, elem_bf16=False,
             wh1_prio=0, p3_mod=0):
    mm_dt = bf16 if elem_bf16 else (f32r if agg_f32r else f32)
    wh_dt = f32r if wh_f32r else f32
    w_dt = bf16 if elem_bf16 else f32
    e8_dt = bf16 if elem_bf16 else f32
    nc = bacc.Bacc("TRN2", target_bir_lowering=False, debug=False,
                   num_devices=N_CORES)
    xT = nc.dram_tensor("xT", [IN_FEAT, N_NODES], wh_dt, kind="ExternalInput").ap()
    xTo = nc.dram_tensor("xTown", [IN_FEAT, ROWS], f32, kind="ExternalInput").ap()
    Wd = nc.dram_tensor("W", [IN_FEAT, F], wh_dt, kind="ExternalInput").ap()
    Wa = nc.dram_tensor("Wa", [IN_FEAT, 16], f32, kind="ExternalInput").ap()
    adjT = nc.dram_tensor("adjT", [N_NODES, ROWS], bf16, kind="ExternalInput").ap()
    out = nc.dram_tensor("out", [ROWS, F], f32, kind="ExternalOutput").ap()

    with tile.TileContext(nc) as tc:
      for rep in range(repeat):
        with tc.tile_pool(name=f"persist{rep}", bufs=1) as per:
            # persistent SBUF (unique tags -> dedicated slots)
            wh1 = [per.tile([128, N_HEADS * E], mm_dt, tag=f"wh1_{jc}", name=f"wh1_{jc}")
                   for jc in range(JCH)]
            adjs = [per.tile([128, ROWS], bf16, tag=f"adj_{jc}", name=f"adj_{jc}")
                    for jc in range(JCH)]
            bsrc = [per.tile([128, ROWS], f32, tag=f"bsrc_{h}", name=f"bsrc_{h}")
                    for h in range(N_HEADS)]
            ev_g = [per.tile([128, 8 * 16], f32, tag=f"ev_g{g}", name=f"ev_g{g}")
                    for g in range(4)]
            evq_g = [per.tile([128, 8 * 8], f32, tag=f"evq_g{g}", name=f"evq_g{g}")
                     for g in range(4)]
            evd_g = [per.tile([128, 8 * 8], f32, tag=f"evd_g{g}", name=f"evd_g{g}")
                     for g in range(4)]
            e8 = [per.tile([128, ROWS], e8_dt, tag=f"e8_{h}", name=f"e8_{h}")
                  for h in range(N_HEADS)]
            outsb = [per.tile([128, F], f32, tag=f"out_{k}", name=f"out_{k}") for k in range(4)]
            espall = per.tile([8, ROWS], f32, tag="espall", name="espall")
            # sel[k, h, p] = (k == h): one [8,128] slice per head is a
            # row-selector lhsT -- matmul(sel_h, espall) both selects row h
            # and broadcasts it across 128 partitions in one K=8 matmul.
            sel = per.tile([8, N_HEADS * 128], f32, tag="sel", name="sel")
            ones128 = per.tile([128, 8], f32, tag="ones128", name="ones128")
            ident = per.tile([128, 128], f32, tag="ident", name="ident")

            nc.gpsimd.memset(ones128[:], 1.0)
            make_identity(nc, ident[:])
            nc.gpsimd.memset(sel[:], 0.0)
            adj_dma_done = False
            nc.gpsimd.affine_select(
                out=sel[:].rearrange("k (h p) -> k h p", h=N_HEADS),
                in_=sel[:].rearrange("k (h p) -> k h p", h=N_HEADS),
                compare_op=OP.not_equal, fill=1.0, base=0,
                pattern=[[-1, N_HEADS], [0, 128]], channel_multiplier=1)

            # ---------------- phase 1: Wh, e-vectors, B_src ----------------
            with tc.tile_pool(name=f"ph1{rep}", bufs=1) as ph1, \
                 tc.tile_pool(name=f"ph1ps{rep}", bufs=2, space="PSUM") as ph1ps:
                xt_sb = [ph1.tile([128, N_NODES], wh_dt, tag=f"xt{k}", name=f"xt{k}")
                         for k in range(KCH)]
                xto_sb = [ph1.tile([128, ROWS], f32, tag=f"xto{k}", name=f"xto{k}")
                          for k in range(KCH)]
                w_sb = [ph1.tile([128, F], wh_dt, tag=f"w{k}", name=f"w{k}") for k in range(KCH)]
                wa_sb = [ph1.tile([128, 16], f32, tag=f"wa{k}", name=f"wa{k}")
                         for k in range(KCH)]
                # small DMAs first (e_src/B_src chain gates phase 2), then
                # xT in column chunks so ev matmuls start early
                for k in range(KCH):
                    sl = slice(k * 128, (k + 1) * 128)
                    nc.sync.dma_start(xto_sb[k][:], xTo[sl, :])
                    nc.sync.dma_start(wa_sb[k][:], Wa[sl, :])
                # interleave xT column chunks with adjT chunks on the
                # sync queue (DMA issue on the gpsimd queue costs ~1us of
                # Pool ENGINE time per descriptor -- 33us total -- while
                # sync-queue issue is free); W rides after the first chunk
                # (it only feeds the Wh matmuls, not the phase-2 ramp)
                for c in range(8):
                    csl = slice(c * 512, (c + 1) * 512)
                    for k in range(KCH):
                        sl = slice(k * 128, (k + 1) * 128)
                        nc.sync.dma_start(xt_sb[k][:][:, csl], xT[sl, csl])
                    for jc in range(4 * c, 4 * c + 4):
                        nc.sync.dma_start(adjs[jc][:],
                                          adjT[jc * 128:(jc + 1) * 128, :])
                    if c == 0:
                        for k in range(KCH):
                            sl = slice(k * 128, (k + 1) * 128)
                            nc.sync.dma_start(w_sb[k][:], Wd[sl, :])

                # e_src for this core's own rows: one [8, ROWS] matmul
                # (depends only on the small xTo/Wa DMAs) so phase 2 can
                # start as soon as B_src is ready.
                esp = ph1ps.tile([8, ROWS], f32, tag="esp", name="esp",
                                 bufs=1)
                for k in range(KCH):
                    nc.tensor.matmul(esp[:], wa_sb[k][:][:, 8:16],
                                     xto_sb[k][:],
                                     start=(k == 0), stop=(k == KCH - 1))
                nc.vector.tensor_copy(espall[:], esp[:])
                for h in range(N_HEADS):
                    bp = ph1ps.tile([128, ROWS], f32, tag="bp", name="bp",
                                    bufs=1)
                    nc.tensor.matmul(bp[:],
                                     sel[:][:, h * 128:(h + 1) * 128],
                                     espall[:], start=True, stop=True)
                    nc.scalar.copy(bsrc[h][:], bp[:])
                    nc.scalar.activation(e8[h][:], bsrc[h][:], AF.Exp,
                                         scale=0.8)

                for jc in range(JCH):
                    jsl = slice(jc * 128, (jc + 1) * 128)
                    evp = ph1ps.tile([128, 16], f32, tag="evp", name="evp")
                    for k in range(KCH):
                        nc.tensor.matmul(
                            evp[:], xt_sb[k][:][:, jsl].bitcast(f32),
                            wa_sb[k][:],
                            start=(k == 0), stop=(k == KCH - 1))
                    if ev_act_copies:
                        nc.scalar.copy(
                            ev_g[jc // 8][:][:, (jc % 8) * 16:(jc % 8 + 1) * 16],
                            evp[:])
                    else:
                        nc.vector.tensor_copy(
                            ev_g[jc // 8][:][:, (jc % 8) * 16:(jc % 8 + 1) * 16],
                            evp[:])
                    if jc % 8 == 7:
                        g = jc // 8
                        nc.scalar.activation(
                            evq_g[g][:].rearrange("p (c h) -> p c h", c=8),
                            ev_g[g][:].rearrange("p (c h) -> p c h",
                                                 c=8)[:, :, 0:8],
                            AF.Exp, scale=0.2)
                        nc.scalar.activation(
                            evd_g[g][:].rearrange("p (c h) -> p c h", c=8),
                            ev_g[g][:].rearrange("p (c h) -> p c h",
                                                 c=8)[:, :, 0:8],
                            AF.Exp, scale=1.0)
                for jc in range(JCH):
                    jsl = slice(jc * 128, (jc + 1) * 128)
                    whp = ph1ps.tile([128, F], f32, tag="whp", name="whp",
                                     bufs=3)
                    for k in range(KCH):
                        nc.tensor.matmul(
                            whp[:],
                            xt_sb[k][:][:, jsl],
                            w_sb[k][:],
                            start=(k == 0), stop=(k == KCH - 1))
                    # Wh -> per-head 65-col blocks, col 64 = 1.0
                    # (copies alternate DVE/ACT to balance load;
                    # Memset can't write f32r)
                    dst = wh1[jc][:].rearrange("p (h e) -> p h e", h=N_HEADS)
                    nc.vector.tensor_copy(
                        dst[:, :, 0:OUT_FEAT],
                        whp[:].rearrange("p (h d) -> p h d", h=N_HEADS))
                    nc.vector.tensor_copy(
                        dst[:, :, OUT_FEAT:E],
                        ones128[:].rearrange("p (e o) -> p e o", o=1))


            # -------- phase 2+3: attention, aggregation, finalize --------
            # two head-groups of 4 so PSUM (8 banks) fits accumulators
            # plus finalize scratch.
            with tc.tile_pool(name=f"work{rep}", bufs=4) as work, \
                 tc.tile_pool(name=f"fin{rep}", bufs=4) as fin:
                for hg in range(2):
                    heads = range(hg * 4, hg * 4 + 4)
                    sA = {}
                    with tc.tile_pool(name=f"acc{rep}_{hg}", bufs=1,
                                      space="PSUM") as accp:
                        acc = {h: accp.tile([E, ROWS], f32, tag=f"acc{h}", name=f"acc{h}")
                               for h in heads}
                        for jc in range(JCH):
                            g, o = jc // 8, jc % 8
                            pi = hg * JCH + jc
                            # pass mix: p1_all of every p1_mod passes run
                            # all four heads on the ACT path; the rest split
                            # 2 ACT + 2 factored-GPSIMD (P3 emitted first so
                            # the GPSIMD pipe fills early).
                            if p3_mod and (pi % p3_mod) == 1:
                                kind = "p3"
                            elif (pi % p1_mod) < p1_all:
                                kind = "p1"
                            else:
                                kind = "mix"
                            hl = list(heads)
                            p1_heads = {"p1": hl, "mix": hl[:2], "p3": []}[kind]
                            p3_heads = {"p1": [], "mix": hl[2:], "p3": hl}[kind]
                            rhs_of = {}
                            for pair0 in range(0, len(p3_heads), 2):
                                # P3: w = E8*exp(d) factored (no ACT): DVE
                                # dual tensor_scalar per head into one
                                # double-width tile, then a single batched
                                # mask-mult on GPSIMD (amortizes its ~400ns
                                # per-op launch overhead)
                                pair = p3_heads[pair0:pair0 + 2]
                                g2 = work.tile([128, 2 * ROWS], mm_dt,
                                               tag="g2", name="g2", bufs=4)
                                for idx, h in enumerate(pair):
                                    qap = evq_g[g][:][:, o * 8 + h:
                                                      o * 8 + h + 1]
                                    dap = evd_g[g][:][:, o * 8 + h:
                                                      o * 8 + h + 1]
                                    nc.vector.tensor_scalar(
                                        g2[:][:, idx * ROWS:(idx + 1) * ROWS],
                                        e8[h][:], dap, qap, OP.mult, OP.max)
                                    rhs_of[h] = g2[:][:, idx * ROWS:
                                                      (idx + 1) * ROWS]
                                nc.gpsimd.tensor_tensor(
                                    g2[:].rearrange("p (b r) -> p b r", b=2),
                                    g2[:].rearrange("p (b r) -> p b r", b=2),
                                    adjs[jc][:].unsqueeze(1).broadcast_to(
                                        [128, 2, ROWS]),
                                    OP.mult)
                            for h in p1_heads:
                                # P1: ACT exp + fused DVE max*mask
                                gt = work.tile([128, ROWS], mm_dt, tag="g",
                                               name="gt", bufs=4)
                                qap = evq_g[g][:][:, o * 8 + h:o * 8 + h + 1]
                                wt = work.tile([128, ROWS], w_dt, tag="w",
                                               name="wt", bufs=4)
                                nc.scalar.activation(
                                    wt[:], bsrc[h][:], AF.Exp,
                                    bias=ev_g[g][:][:, o * 16 + h:
                                                    o * 16 + h + 1],
                                    scale=0.8)
                                nc.vector.scalar_tensor_tensor(
                                    gt[:], wt[:], qap,
                                    adjs[jc][:], OP.max, OP.mult)
                                rhs_of[h] = gt[:]
                            for h in heads:
                                nc.tensor.matmul(
                                    acc[h][:],
                                    wh1[jc][:][:, h * E:(h + 1) * E],
                                    rhs_of[h],
                                    start=(jc == 0), stop=(jc == JCH - 1))
                        for h in heads:
                            sA[h] = fin.tile([E, ROWS], f32, tag=f"sA{h % 4}", name=f"sA{h}", bufs=2)
                            nc.scalar.copy(sA[h][:], acc[h][:])
                    with tc.tile_pool(name=f"tp{rep}_{hg}", bufs=2,
                                      space="PSUM") as tpp:
                        for h in heads:
                            tp = tpp.tile([128, 4 * E], f32, tag="tp", name="tp")
                            rec = fin.tile([128, 4], f32, tag="rec", name="rec", bufs=2)
                            for k in range(4):
                                ksl = slice(k * 128, (k + 1) * 128)
                                nc.tensor.transpose(
                                    tp[:][:, k * E:(k + 1) * E],
                                    sA[h][:][:, ksl], ident[0:E, 0:E])
                            nc.vector.reciprocal(
                                rec[:],
                                tp[:].rearrange("p (k e) -> p k e",
                                                k=4)[:, :, OUT_FEAT:E])
                            for k in range(4):
                                nc.vector.tensor_scalar(
                                    outsb[k][:][:, h * OUT_FEAT:(h + 1) * OUT_FEAT],
                                    tp[:][:, k * E:k * E + OUT_FEAT],
                                    rec[:][:, k:k + 1], 0.0, OP.mult, OP.max)
                    csl = slice(hg * 4 * OUT_FEAT, (hg + 1) * 4 * OUT_FEAT)
                    for k in range(4):
                        nc.sync.dma_start(out[k * 128:(k + 1) * 128, csl],
                                          outsb[k][:][:, csl])

    nc.compile()
    return nc


_NC_CACHE = {}


def get_nc(wh_f32r=True, agg_f32r=True, repeat=1, **kw):
    key = (wh_f32r, agg_f32r, repeat, tuple(sorted(kw.items())))
    if key not in _NC_CACHE:
        _NC_CACHE[key] = build_nc(wh_f32r, agg_f32r, repeat, **kw)
    return _NC_CACHE[key]


def make_in_maps(x, adj, W, a):
    x = np.asarray(x, dtype=np.float32)
    adj = np.asarray(adj, dtype=np.float32)
    W = np.asarray(W, dtype=np.float32)
    a = np.asarray(a, dtype=np.float32)

    xT = np.ascontiguousarray(x.T)                       # [256, 4096]
    a_src = a[:, :OUT_FEAT].astype(np.float64)           # [8, 64]
    a_dst = a[:, OUT_FEAT:].astype(np.float64)
    W3 = W.astype(np.float64).reshape(IN_FEAT, N_HEADS, OUT_FEAT)
    wa_dst = np.einsum("khd,hd->kh", W3, a_dst)          # [256, 8]
    wa_src = np.einsum("khd,hd->kh", W3, a_src)
    Wa = np.concatenate([wa_dst, wa_src], axis=1).astype(np.float32)

    in_maps = []
    for c in range(N_CORES):
        rs = slice(c * ROWS, (c + 1) * ROWS)
        in_maps.append({
            "xT": xT,
            "xTown": np.ascontiguousarray(xT[:, rs]),
            "W": W,
            "Wa": Wa,
            "adjT": np.ascontiguousarray(adj[rs, :].T).astype(ml_dtypes.bfloat16),
        })
    return in_maps


def kernel(x, adj, W, a):
    import os
    exact = os.environ.get("GAT_EXACT", "0") == "1"
    # default: f32r (tf32) matmuls, ~1.4e-3 max rel err, ~200us/core.
    # GAT_EXACT=1: full-fp32 matmuls, ~5e-6 max rel err, ~320us/core.
    nc = get_nc(wh_f32r=not exact, agg_f32r=not exact)
    in_maps = make_in_maps(x, adj, W, a)
    res = run_bass_kernel_spmd(nc, in_maps, core_ids=list(range(N_CORES)))
    return np.concatenate([res.results[c]["out"] for c in range(N_CORES)],
                          axis=0)


if __name__ == "__main__":
    rng = np.random.default_rng(0)
    x = rng.standard_normal((N_NODES, IN_FEAT), dtype=np.float32)
    adj = (rng.random((N_NODES, N_NODES)) < 0.01).astype(np.float32)
    np.fill_diagonal(adj, 1.0)
    W = (rng.standard_normal((IN_FEAT, F), dtype=np.float32) * 0.05)
    a = rng.standard_normal((N_HEADS, 2 * OUT_FEAT), dtype=np.float32)
    out = kernel(x=x, adj=adj, W=W, a=a)
    print("out", out.shape, out.dtype, float(np.abs(out).max()))

